# revision 12
# baseline (speedup 1.0000x reference)
"""GroupedQueryAttention (B=1, S=4096, D=1024, G=16 heads, DH=64) on 8 TRN2 NeuronCores.

Sharding: tensor-parallel over heads. Core c computes heads {2c, 2c+1}:
  - Q/K/V projections with column-sliced weights (128 out-dims per core),
    producing Q^T/K^T/V^T in [dout, seq] fp16 layout (host pre-transposes
    and casts inputs to fp16; all matmuls run at full PE rate).
  - V^T is transposed per 128-key chunk on the PE into key-natural fp8e4
    layout Vnat[key, group, pair, head, dh]; the eviction multiplies by
    the 0/1 mask and column DH holds the mask value itself, so the PV
    matmul accumulates a correctly-masked softmax denominator and exp()
    needs no mask bias.
  - Flash-style attention without max-subtraction: QK^T scores for two
    128-key chunks land in a 2-bank PSUM tile; exp produces fp8e4
    weights — on the scalar engine (exact exp) for most groups and on
    the vector engine (Schraudolph fast-exp: affine + uint8 cast whose
    bit pattern IS the e4m3 encoding) for the rest, so the two engines
    split the softmax work. PV runs in fp8 DoubleRow mode (two 128-key
    chunks per matmul at 2 rows/cycle). QK of group g+1 is emitted
    before PV of group g so the PE never stalls on exp.
  - Normalization: approx reciprocal of the denominator row (copied to
    SBUF on the scalar engine), partition-broadcast (Pool), multiply
    (DVE) into attnT fp16.
  - Output projection with row-sliced Wo; PSUM evicts to fp16 partials
    (4096, 1024) per core; host sums the 8 partials and adds bo.
"""

import os
import sys

for _p in ("/opt/trn_rl_repo", "/root/.axon_site/_ro/trn_rl_repo"):
    if os.path.isdir(_p) and _p not in sys.path:
        sys.path.insert(0, _p)

from contextlib import ExitStack

import ml_dtypes
import numpy as np

import concourse.bass as bass
import concourse.mybir as mybir
import concourse.tile as tile
from concourse import bacc
from concourse.bass_utils import run_bass_kernel_spmd
from concourse.masks import make_identity

S = 4096          # sequence length
D = 1024          # model dim
G = 16            # heads
DH = 64           # head dim
P = 128           # partitions
QT = 512          # attention q-tile (PSUM bank width in fp32)
ST = 1024         # projection s-tile (2 PSUM banks)
KC = 128          # k-chunk (keys per QK stationary)
GK = 2            # k-chunks per exp group (2 banks wide)
NCORES = 8
HPC = G // NCORES             # heads per core = 2
N_QT = S // QT                # 8 attention q-tiles
N_ST = S // ST                # 4 projection s-tiles
N_KCH = D // P                # 8 contraction chunks for projections
N_KC = S // KC                # 32 k-chunks for attention
N_G = N_KC // GK              # 16 exp groups per (qt, h)
DSL = P                       # per-core dout slice (2 heads * 64)
VW = 128                      # Vnat row width (64 dh + mask col + zero pad)

# Schraudolph fast-exp in e4m3 bits: u8 = round(score * EXA + EXB);
# bitcast(u8) ~= exp(score / 8). Calibrated to minimize softmax error.
EXA = 0.125 * 8.0 * 1.4426950408889634
EXB = 55.18

F32 = mybir.dt.float32
F16 = mybir.dt.float16
F8 = mybir.dt.float8e4
U8 = mybir.dt.uint8

_CACHE = {}


def _build_nc(dbg=False):
    key = ("nc", dbg)
    if key in _CACHE:
        return _CACHE[key]

    nc = bacc.Bacc(
        "TRN2", target_bir_lowering=False, debug=False, num_devices=NCORES
    )

    xqT = nc.dram_tensor("xqT", [D, S], F16, kind="ExternalInput").ap()
    xkT = nc.dram_tensor("xkT", [D, S], F16, kind="ExternalInput").ap()
    xvT = nc.dram_tensor("xvT", [D, S], F16, kind="ExternalInput").ap()
    wqT = nc.dram_tensor("wqT", [N_KCH, P, DSL], F16, kind="ExternalInput").ap()
    wkT = nc.dram_tensor("wkT", [N_KCH, P, DSL], F16, kind="ExternalInput").ap()
    wvT = nc.dram_tensor("wvT", [N_KCH, P, DSL], F16, kind="ExternalInput").ap()
    woT = nc.dram_tensor("woT", [DSL, D], F16, kind="ExternalInput").ap()
    bq = nc.dram_tensor("bq", [DSL, 1], F32, kind="ExternalInput").ap()
    bk = nc.dram_tensor("bk", [DSL, 1], F32, kind="ExternalInput").ap()
    bv = nc.dram_tensor("bv", [DSL, 1], F32, kind="ExternalInput").ap()
    m32 = nc.dram_tensor("m32", [P, N_KC], F32, kind="ExternalInput").ap()
    m8 = nc.dram_tensor("m8", [P, N_KC], F8, kind="ExternalInput").ap()
    out_d = nc.dram_tensor("out", [S, D], F16, kind="ExternalOutput").ap()
    if dbg:
        dbg_d = {
            n: nc.dram_tensor(f"dbg_{n}", shp, F32, kind="ExternalOutput").ap()
            for n, shp in (
                ("qts", [P, S]), ("kts", [P, S]), ("vts", [P, S]),
                ("vnat", [P, N_KC * HPC * VW]), ("attnT", [P, S]),
                ("et", [P, S]),
            )
        }

    with tile.TileContext(nc) as tc, ExitStack() as ctx:
        consts = ctx.enter_context(tc.tile_pool(name="consts", bufs=1))
        big = ctx.enter_context(tc.tile_pool(name="big", bufs=1))
        xin = ctx.enter_context(tc.tile_pool(name="xin", bufs=12))
        et_pool = ctx.enter_context(tc.tile_pool(name="et", bufs=4))
        small = ctx.enter_context(tc.tile_pool(name="small", bufs=4))
        ps = ctx.enter_context(tc.tile_pool(name="ps", bufs=2, space="PSUM"))

        # ---- constants ----
        ident = consts.tile([P, P], F16)
        make_identity(nc, ident[:])

        w_s = {}
        for name, wd in (("q", wqT), ("k", wkT), ("v", wvT)):
            w = consts.tile([P, N_KCH * DSL], F16, tag=f"w{name}")
            for kc in range(N_KCH):
                nc.sync.dma_start(w[:, kc * DSL:(kc + 1) * DSL], wd[kc])
            w_s[name] = w
        wo_s = consts.tile([DSL, D], F16, tag="wo")
        nc.sync.dma_start(wo_s[:], woT)
        b_s = {}
        for name, bd in (("q", bq), ("k", bk), ("v", bv)):
            b = consts.tile([DSL, 1], F32, tag=f"b{name}")
            nc.sync.dma_start(b[:], bd)
            b_s[name] = b
        zb = consts.tile([P, 1], F32, tag="zb")
        nc.vector.memset(zb[:], 0.0)
        m32_s = consts.tile([P, N_KC], F32, tag="m32")
        nc.sync.dma_start(m32_s[:], m32)
        m8_s = consts.tile([P, N_G, GK, 1], F8, tag="m8")
        nc.sync.dma_start(m8_s[:, :, :, 0:1], m8)

        # ---- resident activations ----
        QTs = big.tile([P, S], F16, tag="QTs")      # Q^T  [dout, s]
        KTs = big.tile([P, S], F16, tag="KTs")      # K^T  [dout, s]
        VTs = big.tile([P, S], F16, tag="VTs")      # V^T  [dout, s]
        # V natural fp8: [key-part, group, pair, head, VW]; col DH holds the
        # 0/1 mask so PV also accumulates the masked softmax denominator.
        Vnat = big.tile([P, N_G, GK, HPC, VW], F8, tag="Vnat")
        nc.gpsimd.memset(Vnat[:], 0.0)
        attnT = big.tile([P, S], F16, tag="attnT")  # attn output^T [din, s]

        def proj(name, xd, dst, st):
            sl = slice(st * ST, (st + 1) * ST)
            pp = ps.tile([P, ST], F32, tag="qk", name="pp")
            for kc in range(N_KCH):
                xt = xin.tile([P, ST], F16, tag="xt", name="xt")
                nc.sync.dma_start(xt[:], xd[kc * P:(kc + 1) * P, sl])
                for half in range(ST // QT):
                    hsl = slice(half * QT, (half + 1) * QT)
                    nc.tensor.matmul(
                        pp[:, hsl],
                        w_s[name][:, kc * DSL:(kc + 1) * DSL],
                        xt[:, hsl],
                        start=(kc == 0),
                        stop=(kc == N_KCH - 1),
                    )
            nc.scalar.activation(
                dst[:, sl], pp[:],
                mybir.ActivationFunctionType.Identity,
                bias=b_s[name][:], scale=1.0,
            )

        # ---- phase 1: projections ----
        for st in range(N_ST):
            proj("k", xkT, KTs, st)
        proj("q", xqT, QTs, 0)
        for st in range(N_ST):
            proj("v", xvT, VTs, st)
            # transpose this s-tile of V^T into V natural, masking rows
            for h in range(HPC):
                hs = slice(h * DH, (h + 1) * DH)
                for j in range(ST // KC):
                    kc = st * (ST // KC) + j
                    pt = ps.tile([P, DH], F16, tag="op", name="pt")
                    nc.tensor.transpose(
                        pt[:], VTs[hs, kc * KC:(kc + 1) * KC],
                        ident[hs, hs],
                    )
                    nc.vector.tensor_scalar_mul(
                        Vnat[:, kc // GK, kc % GK, h, 0:DH], pt[:],
                        m32_s[:, kc:kc + 1],
                    )
        for h in range(HPC):
            nc.gpsimd.tensor_copy(Vnat[:, :, :, h, DH:DH + 1],
                                  m8_s[:, :, :, 0:1])

        # ---- phase 2+3: attention + output projection ----
        def make_outproj(qt):
            def emit():
                for jq in range(QT // P):
                    qc = qt * (QT // P) + jq
                    for nt in range(D // QT):
                        po = ps.tile([P, QT], F32, tag="op", name="po")
                        nc.tensor.matmul(
                            po[:],
                            attnT[:, qc * P:(qc + 1) * P],
                            wo_s[:, nt * QT:(nt + 1) * QT],
                            start=True, stop=True,
                        )
                        ot = xin.tile([P, QT], F16, tag="ot", name="ot",
                                      bufs=4)
                        if nt == 0:
                            nc.vector.tensor_copy(ot[:], po[:])
                        else:
                            nc.scalar.copy(ot[:], po[:])
                        nc.sync.dma_start(
                            out_d[qc * P:(qc + 1) * P, nt * QT:(nt + 1) * QT],
                            ot[:],
                        )
            return emit

        pending_op = None
        for qt in range(N_QT):
            if qt >= 2 and qt % 2 == 0:
                proj("q", xqT, QTs, qt // 2)
            qsl = slice(qt * QT, (qt + 1) * QT)
            for h in range(HPC):
                hs = slice(h * DH, (h + 1) * DH)
                pv = ps.tile([P, QT], F32, tag="pv", name="pv")

                def qk_group(g):
                    pq = ps.tile([P, ST], F32, tag="qk", name="pq")
                    for j in range(GK):
                        kc = g * GK + j
                        nc.tensor.matmul(
                            pq[:, j * QT:(j + 1) * QT],
                            KTs[hs, kc * KC:(kc + 1) * KC],
                            QTs[hs, qsl],
                            start=True, stop=True,
                        )
                    return pq

                pq = qk_group(0)
                for g in range(N_G):
                    et = et_pool.tile([P, GK, QT], F8, tag="et", name="et")
                    if g % 4 == 3:
                        # Schraudolph fast-exp on DVE: e4m3 bits via affine
                        nc.vector.tensor_scalar(
                            et[:].bitcast(U8), pq[:],
                            EXA, EXB,
                            mybir.AluOpType.mult, mybir.AluOpType.add,
                        )
                    else:
                        nc.scalar.activation(
                            et[:], pq[:],
                            mybir.ActivationFunctionType.Exp,
                            bias=zb[:], scale=0.125,
                        )
                    if g + 1 < N_G:
                        pq = qk_group(g + 1)
                    nc.tensor.matmul(
                        pv[:, :],
                        Vnat[:, g, :, h, 0:VW],
                        et[:, :, :],
                        perf_mode=mybir.MatmulPerfMode.DoubleRow,
                        start=(g == 0), stop=(g == N_G - 1),
                    )
                    if pending_op is not None and h == 0 and g == 4:
                        pending_op()
                        pending_op = None
                # normalize: attnT[hs, qsl] = pv[0:DH] * (1/pv[DH])
                den = small.tile([1, QT], F32, tag="den", name="den")
                nc.scalar.copy(den[:], pv[DH:DH + 1, :])
                rec = small.tile([1, QT], F32, tag="rec", name="rec")
                nc.vector.reciprocal_approx_fast(rec[:], den[:])
                bc = small.tile([DH, QT], F32, tag="bc", name="bc")
                nc.gpsimd.partition_broadcast(bc[:], rec[:])
                nc.vector.tensor_mul(attnT[hs, qsl], pv[0:DH, :], bc[:])
            pending_op = make_outproj(qt)
        pending_op()

        if dbg:
            for name, t in (("qts", QTs), ("kts", KTs), ("vts", VTs),
                            ("attnT", attnT)):
                nc.gpsimd.dma_start(dbg_d[name][:, :], t[:])
            nc.gpsimd.dma_start(dbg_d["vnat"][:, :], Vnat[:])

    nc.compile()
    _CACHE[key] = nc
    return nc


def _prep_in_maps(query, key, value, mask, Wq, bq, Wk, bk, Wv, bv, Wo, bo):
    f = np.float32
    h = np.float16
    f8 = ml_dtypes.float8_e4m3fn
    qT = np.ascontiguousarray(np.asarray(query, dtype=f)[0].T.astype(h))
    kT = np.ascontiguousarray(np.asarray(key, dtype=f)[0].T.astype(h))
    vT = np.ascontiguousarray(np.asarray(value, dtype=f)[0].T.astype(h))
    m01 = (np.asarray(mask)[0] != 0).astype(f)        # [S] 0/1
    m01 = np.ascontiguousarray(m01.reshape(N_KC, KC).T)  # [128, 32]
    WqT, WkT, WvT, WoT = (np.asarray(W, dtype=f).T.astype(h)
                          for W in (Wq, Wk, Wv, Wo))
    in_maps = []
    for c in range(NCORES):
        cs = slice(c * DSL, (c + 1) * DSL)
        in_maps.append({
            "xqT": qT, "xkT": kT, "xvT": vT,
            "wqT": np.ascontiguousarray(WqT[:, cs]).reshape(N_KCH, P, DSL),
            "wkT": np.ascontiguousarray(WkT[:, cs]).reshape(N_KCH, P, DSL),
            "wvT": np.ascontiguousarray(WvT[:, cs]).reshape(N_KCH, P, DSL),
            "woT": np.ascontiguousarray(WoT[cs, :]),
            "bq": np.ascontiguousarray(bq[cs].astype(f, copy=False)).reshape(DSL, 1),
            "bk": np.ascontiguousarray(bk[cs].astype(f, copy=False)).reshape(DSL, 1),
            "bv": np.ascontiguousarray(bv[cs].astype(f, copy=False)).reshape(DSL, 1),
            "m32": m01,
            "m8": m01.astype(f8),
        })
    return in_maps


def run(inputs, trace=False, trace_kwargs=None, dbg=False):
    nc = _build_nc(dbg=dbg)
    in_maps = _prep_in_maps(**inputs)
    res = run_bass_kernel_spmd(
        nc, in_maps, core_ids=list(range(NCORES)), trace=trace,
        **(trace_kwargs or {}),
    )
    bo = np.asarray(inputs["bo"], dtype=np.float32)
    acc = np.zeros((S, D), dtype=np.float32)
    for r in res.results:
        acc += r["out"].astype(np.float32)
    out = (acc + bo[None, :]).astype(np.float32)[None]
    return out, res


def kernel(**inputs):
    out, _ = run(inputs, trace=False)
    return out


# revision 13
# speedup vs baseline: 1.0257x; 1.0257x over previous
"""GroupedQueryAttention (B=1, S=4096, D=1024, G=16 heads, DH=64) on 8 TRN2 NeuronCores.

Sharding: tensor-parallel over heads. Core c computes heads {2c, 2c+1}:
  - Q/K/V projections with column-sliced weights (128 out-dims per core),
    producing Q^T/K^T/V^T in [dout, seq] fp16 layout (host pre-transposes
    and casts inputs to fp16; fp16 matmuls run at full PE rate without
    the power throttling that fp32/fp8-dual modes trigger).
  - V^T is transposed per 128-key chunk on the PE into key-natural fp16
    layout Vnat[key, chunk, head, dh]; the eviction multiplies by the
    0/1 mask and column DH holds the mask value itself, so the PV matmul
    accumulates a correctly-masked softmax denominator and exp() needs
    no mask bias.
  - Flash-style attention without max-subtraction: QK^T scores for two
    128-key chunks land in a 2-bank PSUM tile; one wide exp per group
    produces fp16 weights — exact exp on the scalar engine for 3 of 4
    groups, Schraudolph fast-exp (affine + uint16 cast whose bit pattern
    IS the fp16 encoding) on the vector engine for the rest, so exp
    never gates the PE. QK of group g+1 is emitted before PV of group g.
  - Normalization: approx reciprocal of the denominator row (copied to
    SBUF on the scalar engine), partition-broadcast (Pool), multiply
    (DVE) into attnT fp16.
  - Output projection with row-sliced Wo; PSUM evicts to fp16 partials
    (4096, 1024) per core (split scalar/vector); host sums 8 partials
    and adds bo. Inputs stream on two DMA queues (sync + scalar).
"""

import os
import sys

for _p in ("/opt/trn_rl_repo", "/root/.axon_site/_ro/trn_rl_repo"):
    if os.path.isdir(_p) and _p not in sys.path:
        sys.path.insert(0, _p)

from contextlib import ExitStack

import numpy as np

import concourse.bass as bass
import concourse.mybir as mybir
import concourse.tile as tile
from concourse import bacc
from concourse.bass_utils import run_bass_kernel_spmd
from concourse.masks import make_identity

S = 4096          # sequence length
D = 1024          # model dim
G = 16            # heads
DH = 64           # head dim
P = 128           # partitions
QT = 512          # attention q-tile (PSUM bank width in fp32)
ST = 1024         # projection s-tile (2 PSUM banks)
KC = 128          # k-chunk (keys per QK stationary)
GK = 2            # k-chunks per exp group (2 banks wide)
NCORES = 8
HPC = G // NCORES             # heads per core = 2
N_QT = S // QT                # 8 attention q-tiles
N_ST = S // ST                # 4 projection s-tiles
N_KCH = D // P                # 8 contraction chunks for projections
N_KC = S // KC                # 32 k-chunks for attention
N_G = N_KC // GK              # 16 exp groups per (qt, h)
DSL = P                       # per-core dout slice (2 heads * 64)
VW = DH + 4                   # Vnat row width (64 dh + mask col + pad)

# Schraudolph fast-exp in fp16 bits: u16 = round(score * EXA + EXB);
# bitcast(u16) ~= exp(score / 8). Calibrated to minimize softmax error.
EXA = 0.125 * 1024 * 1.4426950408889634
EXB = 15359.35

F32 = mybir.dt.float32
F16 = mybir.dt.float16
U16 = mybir.dt.uint16

_CACHE = {}


def _build_nc(dbg=False):
    key = ("nc", dbg)
    if key in _CACHE:
        return _CACHE[key]

    nc = bacc.Bacc(
        "TRN2", target_bir_lowering=False, debug=False, num_devices=NCORES
    )

    xqT = nc.dram_tensor("xqT", [D, S], F16, kind="ExternalInput").ap()
    xkT = nc.dram_tensor("xkT", [D, S], F16, kind="ExternalInput").ap()
    xvT = nc.dram_tensor("xvT", [D, S], F16, kind="ExternalInput").ap()
    wqT = nc.dram_tensor("wqT", [N_KCH, P, DSL], F16, kind="ExternalInput").ap()
    wkT = nc.dram_tensor("wkT", [N_KCH, P, DSL], F16, kind="ExternalInput").ap()
    wvT = nc.dram_tensor("wvT", [N_KCH, P, DSL], F16, kind="ExternalInput").ap()
    woT = nc.dram_tensor("woT", [DSL, D], F16, kind="ExternalInput").ap()
    bq = nc.dram_tensor("bq", [DSL, 1], F32, kind="ExternalInput").ap()
    bk = nc.dram_tensor("bk", [DSL, 1], F32, kind="ExternalInput").ap()
    bv = nc.dram_tensor("bv", [DSL, 1], F32, kind="ExternalInput").ap()
    m32 = nc.dram_tensor("m32", [P, N_KC], F32, kind="ExternalInput").ap()
    m16 = nc.dram_tensor("m16", [P, N_KC], F16, kind="ExternalInput").ap()
    out_d = nc.dram_tensor("out", [S, D], F16, kind="ExternalOutput").ap()
    if dbg:
        dbg_d = {
            n: nc.dram_tensor(f"dbg_{n}", shp, F32, kind="ExternalOutput").ap()
            for n, shp in (
                ("qts", [P, S]), ("kts", [P, S]), ("vts", [P, S]),
                ("vnat", [P, N_KC * HPC * VW]), ("attnT", [P, S]),
            )
        }

    with tile.TileContext(nc) as tc, ExitStack() as ctx:
        consts = ctx.enter_context(tc.tile_pool(name="consts", bufs=1))
        big = ctx.enter_context(tc.tile_pool(name="big", bufs=1))
        xin = ctx.enter_context(tc.tile_pool(name="xin", bufs=12))
        et_pool = ctx.enter_context(tc.tile_pool(name="et", bufs=4))
        small = ctx.enter_context(tc.tile_pool(name="small", bufs=4))
        ps = ctx.enter_context(tc.tile_pool(name="ps", bufs=2, space="PSUM"))

        # ---- constants ----
        ident = consts.tile([P, P], F16)
        make_identity(nc, ident[:])

        w_s = {}
        for name, wd in (("q", wqT), ("k", wkT), ("v", wvT)):
            w = consts.tile([P, N_KCH * DSL], F16, tag=f"w{name}")
            for kc in range(N_KCH):
                nc.sync.dma_start(w[:, kc * DSL:(kc + 1) * DSL], wd[kc])
            w_s[name] = w
        wo_s = consts.tile([DSL, D], F16, tag="wo")
        nc.sync.dma_start(wo_s[:], woT)
        b_s = {}
        for name, bd in (("q", bq), ("k", bk), ("v", bv)):
            b = consts.tile([DSL, 1], F32, tag=f"b{name}")
            nc.sync.dma_start(b[:], bd)
            b_s[name] = b
        zb = consts.tile([P, 1], F32, tag="zb")
        nc.vector.memset(zb[:], 0.0)
        m32_s = consts.tile([P, N_KC], F32, tag="m32")
        nc.sync.dma_start(m32_s[:], m32)
        m16_s = consts.tile([P, N_KC, 1], F16, tag="m16")
        nc.sync.dma_start(m16_s[:, :, 0:1], m16)

        # ---- resident activations ----
        QTs = big.tile([P, S], F16, tag="QTs")      # Q^T  [dout, s]
        KTs = big.tile([P, S], F16, tag="KTs")      # K^T  [dout, s]
        VTs = big.tile([P, S], F16, tag="VTs")      # V^T  [dout, s]
        # V natural: [key-part, chunk, head, VW]; col DH holds the 0/1 mask
        # so PV also accumulates the masked softmax denominator.
        Vnat = big.tile([P, N_KC, HPC, VW], F16, tag="Vnat")
        attnT = big.tile([P, S], F16, tag="attnT")  # attn output^T [din, s]

        def proj(name, xd, dst, st, dma):
            sl = slice(st * ST, (st + 1) * ST)
            pp = ps.tile([P, ST], F32, tag="qk", name="pp")
            for kc in range(N_KCH):
                xt = xin.tile([P, ST], F16, tag="xt", name="xt")
                dma.dma_start(xt[:], xd[kc * P:(kc + 1) * P, sl])
                for half in range(ST // QT):
                    hsl = slice(half * QT, (half + 1) * QT)
                    nc.tensor.matmul(
                        pp[:, hsl],
                        w_s[name][:, kc * DSL:(kc + 1) * DSL],
                        xt[:, hsl],
                        start=(kc == 0),
                        stop=(kc == N_KCH - 1),
                    )
            nc.scalar.activation(
                dst[:, sl], pp[:],
                mybir.ActivationFunctionType.Identity,
                bias=b_s[name][:], scale=1.0,
            )

        # ---- phase 1: projections ----
        # xk/xq stream on the sync DMA queue, xv on the scalar queue, so
        # K+Q and V arrive in parallel and attention starts sooner.
        for st in range(N_ST):
            proj("k", xkT, KTs, st, nc.sync)
            proj("v", xvT, VTs, st, nc.scalar)
            # transpose this s-tile of V^T into V natural, masking rows
            for h in range(HPC):
                hs = slice(h * DH, (h + 1) * DH)
                for j in range(ST // KC):
                    kc = st * (ST // KC) + j
                    pt = ps.tile([P, DH], F16, tag="op", name="pt")
                    nc.tensor.transpose(
                        pt[:], VTs[hs, kc * KC:(kc + 1) * KC],
                        ident[hs, hs],
                    )
                    nc.vector.tensor_scalar_mul(
                        Vnat[:, kc, h, 0:DH], pt[:],
                        m32_s[:, kc:kc + 1],
                    )
        proj("q", xqT, QTs, 0, nc.sync)
        for h in range(HPC):
            nc.gpsimd.tensor_copy(Vnat[:, :, h, DH:DH + 1], m16_s[:, :, 0:1])

        # ---- phase 2+3: attention + output projection ----
        def make_outproj(qt):
            def emit():
                for jq in range(QT // P):
                    qc = qt * (QT // P) + jq
                    for nt in range(D // QT):
                        po = ps.tile([P, QT], F32, tag="op", name="po")
                        nc.tensor.matmul(
                            po[:],
                            attnT[:, qc * P:(qc + 1) * P],
                            wo_s[:, nt * QT:(nt + 1) * QT],
                            start=True, stop=True,
                        )
                        ot = xin.tile([P, QT], F16, tag="ot", name="ot",
                                      bufs=4)
                        if nt == 0:
                            nc.vector.tensor_copy(ot[:], po[:])
                        else:
                            nc.scalar.copy(ot[:], po[:])
                        nc.sync.dma_start(
                            out_d[qc * P:(qc + 1) * P, nt * QT:(nt + 1) * QT],
                            ot[:],
                        )
            return emit

        pending_op = None
        for qt in range(N_QT):
            if qt >= 2 and qt % 2 == 0:
                proj("q", xqT, QTs, qt // 2, nc.sync)
            qsl = slice(qt * QT, (qt + 1) * QT)
            for h in range(HPC):
                hs = slice(h * DH, (h + 1) * DH)
                pv = ps.tile([P, QT], F32, tag="pv", name="pv")

                def qk_group(g):
                    pq = ps.tile([P, ST], F32, tag="qk", name="pq")
                    for j in range(GK):
                        kc = g * GK + j
                        nc.tensor.matmul(
                            pq[:, j * QT:(j + 1) * QT],
                            KTs[hs, kc * KC:(kc + 1) * KC],
                            QTs[hs, qsl],
                            start=True, stop=True,
                        )
                    return pq

                pq = qk_group(0)
                for g in range(N_G):
                    et = et_pool.tile([P, GK * QT], F16, tag="et", name="et")
                    if g % 4 == 3:
                        # Schraudolph fast-exp on DVE: fp16 bits via affine
                        nc.vector.tensor_scalar(
                            et[:].bitcast(U16), pq[:],
                            EXA, EXB,
                            mybir.AluOpType.mult, mybir.AluOpType.add,
                        )
                    else:
                        nc.scalar.activation(
                            et[:], pq[:],
                            mybir.ActivationFunctionType.Exp,
                            bias=zb[:], scale=0.125,
                        )
                    if g + 1 < N_G:
                        pq = qk_group(g + 1)
                    for j in range(GK):
                        kc = g * GK + j
                        nc.tensor.matmul(
                            pv[0:DH + 1, :],
                            Vnat[:, kc, h, 0:DH + 1],
                            et[:, j * QT:(j + 1) * QT],
                            start=(kc == 0), stop=(kc == N_KC - 1),
                        )
                    if pending_op is not None and h == 0 and g == 4:
                        pending_op()
                        pending_op = None
                # normalize: attnT[hs, qsl] = pv[0:DH] * (1/pv[DH])
                den = small.tile([1, QT], F32, tag="den", name="den")
                nc.scalar.copy(den[:], pv[DH:DH + 1, :])
                rec = small.tile([1, QT], F32, tag="rec", name="rec")
                nc.vector.reciprocal_approx_fast(rec[:], den[:])
                bc = small.tile([DH, QT], F32, tag="bc", name="bc")
                nc.gpsimd.partition_broadcast(bc[:], rec[:])
                nc.vector.tensor_mul(attnT[hs, qsl], pv[0:DH, :], bc[:])
            pending_op = make_outproj(qt)
        pending_op()

        if dbg:
            for name, t in (("qts", QTs), ("kts", KTs), ("vts", VTs),
                            ("attnT", attnT)):
                nc.gpsimd.dma_start(dbg_d[name][:, :], t[:])
            nc.gpsimd.dma_start(dbg_d["vnat"][:, :], Vnat[:])

    nc.compile()
    _CACHE[key] = nc
    return nc


def _prep_in_maps(query, key, value, mask, Wq, bq, Wk, bk, Wv, bv, Wo, bo):
    f = np.float32
    h = np.float16
    qT = np.ascontiguousarray(np.asarray(query, dtype=f)[0].T.astype(h))
    kT = np.ascontiguousarray(np.asarray(key, dtype=f)[0].T.astype(h))
    vT = np.ascontiguousarray(np.asarray(value, dtype=f)[0].T.astype(h))
    m01 = (np.asarray(mask)[0] != 0).astype(f)        # [S] 0/1
    m01 = np.ascontiguousarray(m01.reshape(N_KC, KC).T)  # [128, 32]
    WqT, WkT, WvT, WoT = (np.asarray(W, dtype=f).T.astype(h)
                          for W in (Wq, Wk, Wv, Wo))
    in_maps = []
    for c in range(NCORES):
        cs = slice(c * DSL, (c + 1) * DSL)
        in_maps.append({
            "xqT": qT, "xkT": kT, "xvT": vT,
            "wqT": np.ascontiguousarray(WqT[:, cs]).reshape(N_KCH, P, DSL),
            "wkT": np.ascontiguousarray(WkT[:, cs]).reshape(N_KCH, P, DSL),
            "wvT": np.ascontiguousarray(WvT[:, cs]).reshape(N_KCH, P, DSL),
            "woT": np.ascontiguousarray(WoT[cs, :]),
            "bq": np.ascontiguousarray(bq[cs].astype(f, copy=False)).reshape(DSL, 1),
            "bk": np.ascontiguousarray(bk[cs].astype(f, copy=False)).reshape(DSL, 1),
            "bv": np.ascontiguousarray(bv[cs].astype(f, copy=False)).reshape(DSL, 1),
            "m32": m01,
            "m16": m01.astype(h),
        })
    return in_maps


def run(inputs, trace=False, trace_kwargs=None, dbg=False):
    nc = _build_nc(dbg=dbg)
    in_maps = _prep_in_maps(**inputs)
    res = run_bass_kernel_spmd(
        nc, in_maps, core_ids=list(range(NCORES)), trace=trace,
        **(trace_kwargs or {}),
    )
    bo = np.asarray(inputs["bo"], dtype=np.float32)
    acc = np.zeros((S, D), dtype=np.float32)
    for r in res.results:
        acc += r["out"].astype(np.float32)
    out = (acc + bo[None, :]).astype(np.float32)[None]
    return out, res


def kernel(**inputs):
    out, _ = run(inputs, trace=False)
    return out
